# revision 47
# baseline (speedup 1.0000x reference)
"""Distributed Trainium2 kernel for nn_ADJ_SecondLayer (gnn_message_passing).

Reference computes, for B=8192 samples (first half RGB-cam, second half IR-cam)
with C=2048 features and L=6 camera nodes:
  - adj [B+L, B+L]: symmetrically-normalized adjacency D^-1/2 G D^-1/2 of the
    bipartite-ish graph G = [[bb, bc], [bc.T, I]] where bb is the half-half
    block-diagonal ones matrix and bc[i,l] = (modality(cams_i) != modality(l)).
  - x_out [B+L, C] = concat(x, tail) where tail[l] is the attention-weighted
    per-camera center (or running_mean for absent cams).

Key structure: every batch row of adj is D_i * T where T is one of two
template vectors (first-half / second-half masked D).  So adj generation is
(almost) pure DMA: broadcast a template into SBUF, scale rows, stream out.

Sharding: data-parallel over batch rows, 1024 rows/core on 8 cores.  The tiny
L=6 segment sums (score denominators + weighted centers) are all-reduced.
All cams-derived metadata (D vectors, one-hot masks, templates) is computed
on host: cams is a 32KB input; every FLOP that touches x runs on device.
"""

import os
import sys

import numpy as np

for _p in ("/opt/trn_rl_repo",):
    if os.path.isdir(_p) and _p not in sys.path:
        sys.path.append(_p)

import concourse.bass as bass
import concourse.mybir as mybir
from concourse import bacc, tile
from concourse.bass_utils import run_bass_kernel_spmd

F32 = mybir.dt.float32
BF16 = mybir.dt.bfloat16

B = 8192
C = 2048
L = 6
NRGB = 4
HALF = B // 2
NCORES = 8
BS = B // NCORES          # 1024 rows per core
P = 128
NB = BS // P              # 8 row-blocks per core
W = B + L                 # 8198 adj width
NC4 = C // 512            # 4 matmul N-chunks of 512

_CACHE = {}


def _build_nc(uniform: bool):
    """Build + compile the 8-core SPMD program.

    uniform=True: every row of a core's adj block is identical (the reference
    layout guarantees it) -- the host folds the row scale into the template and
    adj generation is pure DMA from one SBUF tile.
    uniform=False: rows are scaled per-partition on DVE/ACT before the store.
    """
    key = ("nc", uniform)
    if key in _CACHE:
        return _CACHE[key]

    nc = bacc.Bacc(
        "TRN2",
        target_bir_lowering=False,
        debug=False,
        enable_asserts=False,
        num_devices=NCORES,
    )

    # ---- I/O ----
    x_p = nc.dram_tensor("x", [BS, C], F32, kind="ExternalInput")
    # NOTE: pre-broadcast on host — a stride-0 (to_broadcast) DMA descriptor
    # is NRT_EXEC_UNIT_UNRECOVERABLE on this runtime.
    tmpl_p = nc.dram_tensor("tmpl", [P, W], F32, kind="ExternalInput")
    dloc_p = nc.dram_tensor("dloc", [P, NB], F32, kind="ExternalInput")
    bcd_p = nc.dram_tensor("bcd", [P, NB * L], F32, kind="ExternalInput")
    mb_p = nc.dram_tensor("mb", [P, NB * L], F32, kind="ExternalInput")
    mt_p = nc.dram_tensor("mt", [L, BS], F32, kind="ExternalInput")
    bsel_p = nc.dram_tensor("bsel", [P, NB], F32, kind="ExternalInput")
    attw_p = nc.dram_tensor("attw", [L, C], F32, kind="ExternalInput")
    tailrows_p = nc.dram_tensor("tailrows", [L, W], F32, kind="ExternalInput")

    adj_p = nc.dram_tensor("adj_rows", [BS, W], F32, kind="ExternalOutput")
    adjt_p = nc.dram_tensor("adj_tail", [L, W], F32, kind="ExternalOutput")
    xout_p = nc.dram_tensor("xout", [BS, C], F32, kind="ExternalOutput")
    # per-core partial segment sums (numer | denom); the 8-way cross-core sum
    # (393KB total) happens on host during unshard -- the on-device AllReduce
    # of this payload costs ~80us of pure latency and sat on the critical path
    part_p = nc.dram_tensor("part", [L, C + 1], F32, kind="ExternalOutput")

    with tile.TileContext(nc) as tc:
        with (
            tc.tile_pool(name="static", bufs=1) as sp,
            tc.tile_pool(name="adjout", bufs=2) as op,
            tc.tile_pool(name="small", bufs=2) as smp,
            tc.tile_pool(name="psum_aw", bufs=2, space="PSUM") as paw,
            tc.tile_pool(name="psum_acc", bufs=1, space="PSUM") as pac,
        ):
            # ---- static loads ----
            # x tiles first, split across both HWDGE rings, so the scores
            # pipeline (and the laggy AllReduce behind it) starts ASAP; then
            # the template; the adj/xout stores stream behind on both rings.
            Tb = sp.tile([P, W], F32)
            nc.sync.dma_start(Tb[:], tmpl_p[:])
            # one DMA for the whole x shard: partition p, free-col block b
            # holds row b*128+p -- fewer ring entries means no HWDGE
            # semaphore-lane ping-pong against the adj stream
            xall = sp.tile([P, NB * C], F32)
            x_v = x_p[:].rearrange("(b p) c -> p b c", p=P)
            nc.scalar.dma_start(xall[:].rearrange("p (b c) -> p b c", b=NB), x_v)
            mt_sb = sp.tile([L, BS], F32)
            nc.gpsimd.dma_start(mt_sb[:], mt_p[:])
            attw_sb = sp.tile([L, C], F32)
            nc.gpsimd.dma_start(attw_sb[:], attw_p[:])
            # bf16 copies for the one-hot gather matmul (one-hot is exact in
            # bf16; att_w rounding only perturbs the tiny x_out tail rows)
            mt_bf = sp.tile([L, BS], BF16)
            nc.vector.tensor_copy(mt_bf[:], mt_sb[:])
            attw_bf = sp.tile([L, C], BF16)
            nc.vector.tensor_copy(attw_bf[:], attw_sb[:])
            mb_sb = sp.tile([P, NB * L], F32)
            nc.gpsimd.dma_start(mb_sb[:], mb_p[:])
            ones_sb = sp.tile([P, 1], F32)
            nc.vector.memset(ones_sb[:], 1.0)
            # Per-partition scalar operands must be dedicated offset-0 [P,1]
            # tiles: this runtime mis-lowers scalar APs with a free-dim offset.
            bselb = []
            for b in range(NB):
                t = sp.tile([P, 1], F32, name=f"bselb{b}")
                nc.gpsimd.dma_start(t[:], bsel_p[:, b : b + 1])
                bselb.append(t)
            if not uniform:
                bcd_sb = sp.tile([P, NB * L], F32)
                nc.gpsimd.dma_start(bcd_sb[:], bcd_p[:])
                dlocb = []
                for b in range(NB):
                    t = sp.tile([P, 1], F32, name=f"dlocb{b}")
                    nc.gpsimd.dma_start(t[:], dloc_p[:, b : b + 1])
                    dlocb.append(t)

            # ---- adj tail passthrough (host-computed, cams-only data) ----
            nc.gpsimd.dma_start(adjt_p[:], tailrows_p[:])

            # ---- per-core accumulators ----
            numer_ps = pac.tile([L, C], F32)     # sum_i M[i,l] * s_i * x_i
            denom_ps = pac.tile([L, 1], F32)     # sum_i M[i,l] * s_i

            for b in range(NB):
                rows = slice(b * P, (b + 1) * P)
                cols = slice(b * L, (b + 1) * L)

                # ---------- adjacency rows ----------
                store_eng = nc.sync
                if uniform:
                    # all rows identical: stream the template straight out
                    store_eng.dma_start(adj_p[rows, :], Tb[:])
                else:
                    ot = op.tile([P, B], F32)
                    if b % 2 == 0:
                        nc.vector.tensor_scalar_mul(ot[:], Tb[:, 0:B], dlocb[b][:])
                    else:
                        nc.scalar.mul(ot[:], Tb[:, 0:B], dlocb[b][:])
                    oc = op.tile([P, L], F32, tag="oc")
                    nc.vector.tensor_scalar_mul(oc[:], bcd_sb[:, cols], dlocb[b][:])
                    store_eng.dma_start(adj_p[rows, 0:B], ot[:])
                    store_eng.dma_start(adj_p[rows, B:W], oc[:])

                xb = xall[:, b * C : (b + 1) * C]

                # ---------- scores: s_i = x_i . att_w[cams_i] + att_b[cams_i] ----------
                # (tensor_tensor_reduce is unsupported on this runtime: use
                #  tensor_mul + tensor_reduce per 512-chunk, then sum partials)
                spart = smp.tile([P, NC4], F32, tag="spart", name=f"spart{b}")
                for c4 in range(NC4):
                    nsl = slice(c4 * 512, (c4 + 1) * 512)
                    awp = paw.tile([P, 512], F32, tag="awp", name=f"awp{b}_{c4}")
                    # gather att_w rows via one-hot matmul: [6,128].T @ [6,512]
                    nc.tensor.matmul(
                        awp[:], mt_bf[:, rows], attw_bf[:, nsl], start=True, stop=True
                    )
                    junk = smp.tile([P, 512], F32, tag="junk", name=f"junk{b}_{c4}")
                    nc.vector.tensor_mul(junk[:], xb[:, nsl], awp[:])
                    nc.vector.tensor_reduce(
                        spart[:, c4 : c4 + 1], junk[:],
                        axis=mybir.AxisListType.X, op=mybir.AluOpType.add,
                    )
                s_red = smp.tile([P, 1], F32, tag="sred", name=f"sred{b}")
                nc.vector.tensor_reduce(
                    s_red[:], spart[:],
                    axis=mybir.AxisListType.X, op=mybir.AluOpType.add,
                )
                s_t = smp.tile([P, 1], F32, tag="st", name=f"st{b}")
                nc.vector.tensor_add(s_t[:], s_red[:], bselb[b][:])

                # ---------- A[i,l] = M[i,l] * s_i ; accumulate numer/denom ----------
                A = smp.tile([P, L], F32, tag="A", name=f"A{b}")
                nc.vector.tensor_scalar_mul(A[:], mb_sb[:, cols], s_t[:])
                first, last = (b == 0), (b == NB - 1)
                for c4 in range(NC4):
                    nsl = slice(c4 * 512, (c4 + 1) * 512)
                    nc.tensor.matmul(
                        numer_ps[:, nsl], A[:], xb[:, nsl], start=first, stop=last
                    )
                nc.tensor.matmul(
                    denom_ps[:], A[:], ones_sb[:], start=first, stop=last
                )

            # ---- x passthrough store, behind the load on the scalar ring ----
            nc.scalar.dma_start(
                xout_p[:].rearrange("(b p) c -> p b c", p=P),
                xall[:].rearrange("p (b c) -> p b c", b=NB),
            )

            # ---- ship per-core partial segment sums; host reduces them ----
            ncat = sp.tile([L, C], F32)
            nc.vector.tensor_copy(ncat[:], numer_ps[:])
            dend = sp.tile([L, 1], F32)
            nc.vector.tensor_copy(dend[:], denom_ps[:])
            nc.gpsimd.dma_start(part_p[:, 0:C], ncat[:])
            nc.gpsimd.dma_start(part_p[:, C : C + 1], dend[:])

    nc.compile()
    _CACHE[key] = nc
    return nc


def _host_prep(x, cams, att_w, att_b, running_mean):
    """All cams/param-derived metadata (cams is a 32KB input; cheap on host)."""
    cams = np.asarray(cams).astype(np.int64).ravel()
    x = np.ascontiguousarray(np.asarray(x), dtype=np.float32)
    att_w = np.ascontiguousarray(np.asarray(att_w), dtype=np.float32)
    att_b = np.asarray(att_b).astype(np.float32).ravel()
    rmean = np.ascontiguousarray(np.asarray(running_mean), dtype=np.float32)

    is_rgb_s = cams < NRGB                               # sample-cam modality
    D = (np.where(is_rgb_s, HALF + 2.0, HALF + 4.0).astype(np.float32)) ** -0.5
    cnt_rgb = int(is_rgb_s.sum())
    rs_c = np.where(np.arange(L) < NRGB, B - cnt_rgb, cnt_rgb).astype(np.float32) + 1.0
    Dc = rs_c ** -0.5
    halfmask = np.arange(B) < HALF
    is_rgb_cam = np.arange(L) < NRGB
    cross = (is_rgb_s[:, None] != is_rgb_cam[None, :])   # [B, L]
    bcD = cross.astype(np.float32) * Dc[None, :]         # [B, L] (un-rowscaled)
    M = (cams[:, None] == np.arange(L)[None, :]).astype(np.float32)
    bsel = att_b[cams]
    counts = np.bincount(cams, minlength=L).astype(np.float32)[:L]
    mask2 = np.stack([(counts > 0).astype(np.float32),
                      (counts == 0).astype(np.float32)], axis=1)
    tail_b = (is_rgb_s[None, :] != is_rgb_cam[:, None]) * (Dc[:, None] * D[None, :])
    tail_rows = np.concatenate(
        [tail_b.astype(np.float32), np.diag(Dc * Dc).astype(np.float32)], axis=1
    )

    per_core = []
    uniform = True
    for k in range(NCORES):
        sl = slice(k * BS, (k + 1) * BS)
        rs = D[sl]                                       # [1024] row scales
        first_half = k < NCORES // 2                     # rows structurally in half
        T = np.where(halfmask == first_half, D, 0.0).astype(np.float32)
        bcD_k = bcD[sl]
        uni_k = bool((rs == rs[0]).all() and (bcD_k == bcD_k[0]).all())
        uniform = uniform and uni_k
        per_core.append((sl, rs, T, bcD_k))

    in_maps = []
    for k in range(NCORES):
        sl, rs, T, bcD_k = per_core[k]
        if uniform:
            tmpl = np.concatenate([T * rs[0], bcD_k[0] * rs[0]]).astype(np.float32)
        else:
            tmpl = np.concatenate([T, np.zeros(L, np.float32)]).astype(np.float32)
        m = {
            "x": x[sl],
            "tmpl": np.ascontiguousarray(np.broadcast_to(tmpl[None, :], (P, W))),
            "dloc": np.ascontiguousarray(rs.reshape(NB, P).T),
            "bcd": np.ascontiguousarray(
                bcD_k.reshape(NB, P, L).transpose(1, 0, 2).reshape(P, NB * L)
            ),
            "mb": np.ascontiguousarray(
                M[sl].reshape(NB, P, L).transpose(1, 0, 2).reshape(P, NB * L)
            ),
            "mt": np.ascontiguousarray(M[sl].T),
            "bsel": np.ascontiguousarray(bsel[sl].reshape(NB, P).T),
            "attw": att_w,
            "tailrows": np.ascontiguousarray(tail_rows),
        }
        in_maps.append(m)
    present = counts > 0
    return in_maps, uniform, (present, rmean)


def _run(inputs, trace=False, tmpdir=None):
    in_maps, uniform, (present, rmean) = _host_prep(
        inputs["x"], inputs["cams"], inputs["att_w"], inputs["att_b"],
        inputs["running_mean"],
    )
    nc = _build_nc(uniform)
    res = run_bass_kernel_spmd(
        nc, in_maps, core_ids=list(range(NCORES)), trace=trace, tmpdir=tmpdir
    )
    results = res.results
    x_out = np.empty((B + L, C), np.float32)
    adj = np.empty((B + L, B + L), np.float32)
    for k in range(NCORES):
        x_out[k * BS : (k + 1) * BS] = results[k]["xout"]
        adj[k * BS : (k + 1) * BS] = results[k]["adj_rows"]
    # unshard the segment sums: 8-way add of [6, 2049] partials + select
    part = np.sum([results[k]["part"] for k in range(NCORES)], axis=0,
                  dtype=np.float32)
    numer, denom = part[:, 0:C], part[:, C]
    tail = np.where(present[:, None], numer / np.where(present, denom, 1.0)[:, None],
                    rmean).astype(np.float32)
    x_out[B:] = tail
    adj[B:] = results[0]["adj_tail"]
    return (x_out, adj), res


def kernel(**inputs):
    out, _ = _run(inputs, trace=False)
    return out


# revision 52
# speedup vs baseline: 1.2080x; 1.2080x over previous
"""Distributed Trainium2 kernel for nn_ADJ_SecondLayer (gnn_message_passing).

Reference computes, for B=8192 samples (first half RGB-cam, second half IR-cam)
with C=2048 features and L=6 camera nodes:
  - adj [B+L, B+L]: symmetrically-normalized adjacency D^-1/2 G D^-1/2 of the
    bipartite-ish graph G = [[bb, bc], [bc.T, I]] where bb is the half-half
    block-diagonal ones matrix and bc[i,l] = (modality(cams_i) != modality(l)).
  - x_out [B+L, C] = concat(x, tail) where tail[l] is the attention-weighted
    per-camera center (or running_mean for absent cams).

Key structure: every batch row of adj is D_i * T where T is one of two
template vectors (first-half / second-half masked D).  So adj generation is
(almost) pure DMA: broadcast a template into SBUF, scale rows, stream out.

Sharding: data-parallel over batch rows, 1024 rows/core on 8 cores.  The tiny
L=6 segment sums (score denominators + weighted centers) are all-reduced.
All cams-derived metadata (D vectors, one-hot masks, templates) is computed
on host: cams is a 32KB input; every FLOP that touches x runs on device.
"""

import os
import sys

import numpy as np

for _p in ("/opt/trn_rl_repo",):
    if os.path.isdir(_p) and _p not in sys.path:
        sys.path.append(_p)

import concourse.bass as bass
import concourse.mybir as mybir
from concourse import bacc, tile
from concourse.bass_utils import run_bass_kernel_spmd
from concourse.tile import add_dep_helper

F32 = mybir.dt.float32
BF16 = mybir.dt.bfloat16

B = 8192
C = 2048
L = 6
NRGB = 4
HALF = B // 2
NCORES = 8
BS = B // NCORES          # 1024 rows per core
P = 128
NB = BS // P              # 8 row-blocks per core
W = B + L                 # 8198 adj width
NC4 = C // 512            # 4 matmul N-chunks of 512

_CACHE = {}


def _build_nc(uniform: bool):
    """Build + compile the 8-core SPMD program.

    uniform=True: every row of a core's adj block is identical (the reference
    layout guarantees it) -- the host folds the row scale into the template and
    adj generation is pure DMA from one SBUF tile.
    uniform=False: rows are scaled per-partition on DVE/ACT before the store.
    """
    key = ("nc", uniform)
    if key in _CACHE:
        return _CACHE[key]

    nc = bacc.Bacc(
        "TRN2",
        target_bir_lowering=False,
        debug=False,
        enable_asserts=False,
        num_devices=NCORES,
    )

    # ---- I/O ----
    x_p = nc.dram_tensor("x", [BS, C], F32, kind="ExternalInput")
    # NOTE: pre-broadcast on host — a stride-0 (to_broadcast) DMA descriptor
    # is NRT_EXEC_UNIT_UNRECOVERABLE on this runtime.
    tmpl_p = nc.dram_tensor("tmpl", [P, W], F32, kind="ExternalInput")
    dloc_p = nc.dram_tensor("dloc", [P, NB], F32, kind="ExternalInput")
    bcd_p = nc.dram_tensor("bcd", [P, NB * L], F32, kind="ExternalInput")
    mb_p = nc.dram_tensor("mb", [P, NB * L], F32, kind="ExternalInput")
    mt_p = nc.dram_tensor("mt", [L, BS], F32, kind="ExternalInput")
    bsel_p = nc.dram_tensor("bsel", [P, NB], F32, kind="ExternalInput")
    attw_p = nc.dram_tensor("attw", [L, C], F32, kind="ExternalInput")
    tailrows_p = nc.dram_tensor("tailrows", [L, W], F32, kind="ExternalInput")

    adj_p = nc.dram_tensor("adj_rows", [BS, W], F32, kind="ExternalOutput")
    adjt_p = nc.dram_tensor("adj_tail", [L, W], F32, kind="ExternalOutput")
    xout_p = nc.dram_tensor("xout", [BS, C], F32, kind="ExternalOutput")
    # per-core partial segment sums (numer | denom); the 8-way cross-core sum
    # (393KB total) happens on host during unshard -- the on-device AllReduce
    # of this payload costs ~80us of pure latency and sat on the critical path
    part_p = nc.dram_tensor("part", [L, C + 1], F32, kind="ExternalOutput")

    with tile.TileContext(nc) as tc:
        with (
            tc.tile_pool(name="static", bufs=1) as sp,
            tc.tile_pool(name="adjout", bufs=2) as op,
            tc.tile_pool(name="small", bufs=2) as smp,
            tc.tile_pool(name="psum_aw", bufs=2, space="PSUM") as paw,
            tc.tile_pool(name="psum_acc", bufs=1, space="PSUM") as pac,
        ):
            # ---- static loads ----
            # x tiles first, split across both HWDGE rings, so the scores
            # pipeline (and the laggy AllReduce behind it) starts ASAP; then
            # the template; the adj/xout stores stream behind on both rings.
            Tb = sp.tile([P, W], F32)
            nc.sync.dma_start(Tb[:], tmpl_p[:])
            # one DMA for the whole x shard: partition p, free-col block b
            # holds row b*128+p -- fewer ring entries means no HWDGE
            # semaphore-lane ping-pong against the adj stream
            xall = sp.tile([P, NB * C], F32)
            x_v = x_p[:].rearrange("(b p) c -> p b c", p=P)
            xload_inst = nc.scalar.dma_start(
                xall[:].rearrange("p (b c) -> p b c", b=NB), x_v
            )
            mt_sb = sp.tile([L, BS], F32)
            nc.gpsimd.dma_start(mt_sb[:], mt_p[:])
            attw_sb = sp.tile([L, C], F32)
            nc.gpsimd.dma_start(attw_sb[:], attw_p[:])
            # bf16 copies for the one-hot gather matmul (one-hot is exact in
            # bf16; att_w rounding only perturbs the tiny x_out tail rows)
            mt_bf = sp.tile([L, BS], BF16)
            nc.vector.tensor_copy(mt_bf[:], mt_sb[:])
            attw_bf = sp.tile([L, C], BF16)
            nc.vector.tensor_copy(attw_bf[:], attw_sb[:])
            mb_sb = sp.tile([P, NB * L], F32)
            nc.gpsimd.dma_start(mb_sb[:], mb_p[:])
            ones_sb = sp.tile([P, 1], F32)
            nc.vector.memset(ones_sb[:], 1.0)
            # Per-partition scalar operands must be dedicated offset-0 [P,1]
            # tiles: this runtime mis-lowers scalar APs with a free-dim offset.
            bselb = []
            for b in range(NB):
                t = sp.tile([P, 1], F32, name=f"bselb{b}")
                nc.gpsimd.dma_start(t[:], bsel_p[:, b : b + 1])
                bselb.append(t)
            if not uniform:
                bcd_sb = sp.tile([P, NB * L], F32)
                nc.gpsimd.dma_start(bcd_sb[:], bcd_p[:])
                dlocb = []
                for b in range(NB):
                    t = sp.tile([P, 1], F32, name=f"dlocb{b}")
                    nc.gpsimd.dma_start(t[:], dloc_p[:, b : b + 1])
                    dlocb.append(t)

            # ---- adj tail passthrough (host-computed, cams-only data) ----
            nc.gpsimd.dma_start(adjt_p[:], tailrows_p[:])

            # ---- per-core accumulators ----
            numer_ps = pac.tile([L, C], F32)     # sum_i M[i,l] * s_i * x_i
            denom_ps = pac.tile([L, 1], F32)     # sum_i M[i,l] * s_i

            for b in range(NB):
                rows = slice(b * P, (b + 1) * P)
                cols = slice(b * L, (b + 1) * L)

                # ---------- adjacency rows ----------
                store_eng = nc.sync
                if uniform:
                    # all rows identical: stream the template straight out
                    adj_inst = store_eng.dma_start(adj_p[rows, :], Tb[:])
                    if b == 0:
                        # two-phase schedule: let the x load run at full rate
                        # before the adj stream starts hogging DMA packets
                        add_dep_helper(
                            adj_inst.ins, xload_inst.ins, sync=True,
                            reason="x load drains before adj stream",
                        )
                else:
                    ot = op.tile([P, B], F32)
                    if b % 2 == 0:
                        nc.vector.tensor_scalar_mul(ot[:], Tb[:, 0:B], dlocb[b][:])
                    else:
                        nc.scalar.mul(ot[:], Tb[:, 0:B], dlocb[b][:])
                    oc = op.tile([P, L], F32, tag="oc")
                    nc.vector.tensor_scalar_mul(oc[:], bcd_sb[:, cols], dlocb[b][:])
                    adj_inst = store_eng.dma_start(adj_p[rows, 0:B], ot[:])
                    if b == 0:
                        add_dep_helper(
                            adj_inst.ins, xload_inst.ins, sync=True,
                            reason="x load drains before adj stream",
                        )
                    store_eng.dma_start(adj_p[rows, B:W], oc[:])

                xb = xall[:, b * C : (b + 1) * C]

                # ---------- scores: s_i = x_i . att_w[cams_i] + att_b[cams_i] ----------
                # (tensor_tensor_reduce is unsupported on this runtime: use
                #  tensor_mul + tensor_reduce per 512-chunk, then sum partials)
                spart = smp.tile([P, NC4], F32, tag="spart", name=f"spart{b}")
                for c4 in range(NC4):
                    nsl = slice(c4 * 512, (c4 + 1) * 512)
                    awp = paw.tile([P, 512], F32, tag="awp", name=f"awp{b}_{c4}")
                    # gather att_w rows via one-hot matmul: [6,128].T @ [6,512]
                    nc.tensor.matmul(
                        awp[:], mt_bf[:, rows], attw_bf[:, nsl], start=True, stop=True
                    )
                    junk = smp.tile([P, 512], F32, tag="junk", name=f"junk{b}_{c4}")
                    nc.vector.tensor_mul(junk[:], xb[:, nsl], awp[:])
                    nc.vector.tensor_reduce(
                        spart[:, c4 : c4 + 1], junk[:],
                        axis=mybir.AxisListType.X, op=mybir.AluOpType.add,
                    )
                s_red = smp.tile([P, 1], F32, tag="sred", name=f"sred{b}")
                nc.vector.tensor_reduce(
                    s_red[:], spart[:],
                    axis=mybir.AxisListType.X, op=mybir.AluOpType.add,
                )
                s_t = smp.tile([P, 1], F32, tag="st", name=f"st{b}")
                nc.vector.tensor_add(s_t[:], s_red[:], bselb[b][:])

                # ---------- A[i,l] = M[i,l] * s_i ; accumulate numer/denom ----------
                A = smp.tile([P, L], F32, tag="A", name=f"A{b}")
                nc.vector.tensor_scalar_mul(A[:], mb_sb[:, cols], s_t[:])
                first, last = (b == 0), (b == NB - 1)
                for c4 in range(NC4):
                    nsl = slice(c4 * 512, (c4 + 1) * 512)
                    nc.tensor.matmul(
                        numer_ps[:, nsl], A[:], xb[:, nsl], start=first, stop=last
                    )
                nc.tensor.matmul(
                    denom_ps[:], A[:], ones_sb[:], start=first, stop=last
                )

            # ---- x passthrough store, behind the load on the scalar ring ----
            nc.scalar.dma_start(
                xout_p[:].rearrange("(b p) c -> p b c", p=P),
                xall[:].rearrange("p (b c) -> p b c", b=NB),
            )

            # ---- ship per-core partial segment sums; host reduces them ----
            ncat = sp.tile([L, C], F32)
            nc.vector.tensor_copy(ncat[:], numer_ps[:])
            dend = sp.tile([L, 1], F32)
            nc.vector.tensor_copy(dend[:], denom_ps[:])
            nc.gpsimd.dma_start(part_p[:, 0:C], ncat[:])
            nc.gpsimd.dma_start(part_p[:, C : C + 1], dend[:])

    nc.compile()
    _CACHE[key] = nc
    return nc


def _host_prep(x, cams, att_w, att_b, running_mean):
    """All cams/param-derived metadata (cams is a 32KB input; cheap on host)."""
    cams = np.asarray(cams).astype(np.int64).ravel()
    x = np.ascontiguousarray(np.asarray(x), dtype=np.float32)
    att_w = np.ascontiguousarray(np.asarray(att_w), dtype=np.float32)
    att_b = np.asarray(att_b).astype(np.float32).ravel()
    rmean = np.ascontiguousarray(np.asarray(running_mean), dtype=np.float32)

    is_rgb_s = cams < NRGB                               # sample-cam modality
    D = (np.where(is_rgb_s, HALF + 2.0, HALF + 4.0).astype(np.float32)) ** -0.5
    cnt_rgb = int(is_rgb_s.sum())
    rs_c = np.where(np.arange(L) < NRGB, B - cnt_rgb, cnt_rgb).astype(np.float32) + 1.0
    Dc = rs_c ** -0.5
    halfmask = np.arange(B) < HALF
    is_rgb_cam = np.arange(L) < NRGB
    cross = (is_rgb_s[:, None] != is_rgb_cam[None, :])   # [B, L]
    bcD = cross.astype(np.float32) * Dc[None, :]         # [B, L] (un-rowscaled)
    M = (cams[:, None] == np.arange(L)[None, :]).astype(np.float32)
    bsel = att_b[cams]
    counts = np.bincount(cams, minlength=L).astype(np.float32)[:L]
    mask2 = np.stack([(counts > 0).astype(np.float32),
                      (counts == 0).astype(np.float32)], axis=1)
    tail_b = (is_rgb_s[None, :] != is_rgb_cam[:, None]) * (Dc[:, None] * D[None, :])
    tail_rows = np.concatenate(
        [tail_b.astype(np.float32), np.diag(Dc * Dc).astype(np.float32)], axis=1
    )

    per_core = []
    uniform = True
    for k in range(NCORES):
        sl = slice(k * BS, (k + 1) * BS)
        rs = D[sl]                                       # [1024] row scales
        first_half = k < NCORES // 2                     # rows structurally in half
        T = np.where(halfmask == first_half, D, 0.0).astype(np.float32)
        bcD_k = bcD[sl]
        uni_k = bool((rs == rs[0]).all() and (bcD_k == bcD_k[0]).all())
        uniform = uniform and uni_k
        per_core.append((sl, rs, T, bcD_k))

    in_maps = []
    for k in range(NCORES):
        sl, rs, T, bcD_k = per_core[k]
        if uniform:
            tmpl = np.concatenate([T * rs[0], bcD_k[0] * rs[0]]).astype(np.float32)
        else:
            tmpl = np.concatenate([T, np.zeros(L, np.float32)]).astype(np.float32)
        m = {
            "x": x[sl],
            "tmpl": np.ascontiguousarray(np.broadcast_to(tmpl[None, :], (P, W))),
            "dloc": np.ascontiguousarray(rs.reshape(NB, P).T),
            "bcd": np.ascontiguousarray(
                bcD_k.reshape(NB, P, L).transpose(1, 0, 2).reshape(P, NB * L)
            ),
            "mb": np.ascontiguousarray(
                M[sl].reshape(NB, P, L).transpose(1, 0, 2).reshape(P, NB * L)
            ),
            "mt": np.ascontiguousarray(M[sl].T),
            "bsel": np.ascontiguousarray(bsel[sl].reshape(NB, P).T),
            "attw": att_w,
            "tailrows": np.ascontiguousarray(tail_rows),
        }
        in_maps.append(m)
    present = counts > 0
    return in_maps, uniform, (present, rmean)


def _run(inputs, trace=False, tmpdir=None):
    in_maps, uniform, (present, rmean) = _host_prep(
        inputs["x"], inputs["cams"], inputs["att_w"], inputs["att_b"],
        inputs["running_mean"],
    )
    nc = _build_nc(uniform)
    res = run_bass_kernel_spmd(
        nc, in_maps, core_ids=list(range(NCORES)), trace=trace, tmpdir=tmpdir
    )
    results = res.results
    x_out = np.empty((B + L, C), np.float32)
    adj = np.empty((B + L, B + L), np.float32)
    for k in range(NCORES):
        x_out[k * BS : (k + 1) * BS] = results[k]["xout"]
        adj[k * BS : (k + 1) * BS] = results[k]["adj_rows"]
    # unshard the segment sums: 8-way add of [6, 2049] partials + select
    part = np.sum([results[k]["part"] for k in range(NCORES)], axis=0,
                  dtype=np.float32)
    numer, denom = part[:, 0:C], part[:, C]
    tail = np.where(present[:, None], numer / np.where(present, denom, 1.0)[:, None],
                    rmean).astype(np.float32)
    x_out[B:] = tail
    adj[B:] = results[0]["adj_tail"]
    return (x_out, adj), res


def kernel(**inputs):
    out, _ = _run(inputs, trace=False)
    return out
